# revision 1
# baseline (speedup 1.0000x reference)
"""LocalWindowAttention Trainium2 kernel (Bass/Tile), 8-core SPMD.

Problem: x[B=4, S=4096, E=512] -> out[B, S, E]
  qkv = x @ W_qkv + b_qkv ; q,k,v = split(qkv)
  scores = (q @ k.T) / sqrt(E), banded mask |i-j| <= 64, softmax
  out = (attn @ v) @ W_out + b_out

Sharding: 8 cores = (batch b in 0..3) x (seq half h in 0..1). Each core owns
2048 query rows and loads a 64-row halo of x on each side (zero-padded at
sequence boundaries), computing q/k/v locally — no collectives.

Per-core layout strategy:
  - x is transposed on host to xT [E, 2176] so the E-contraction of every
    matmul has E on the partition dim.
  - qT/kT produced as [E, rows] (feature on partitions), v as [rows, E].
  - scores tile per 128-query subtile: [128 q, 256 keys] (key window of a
    128-aligned query tile is exactly 256 keys starting 64 left).
  - softmax without max-subtraction (scores are O(1) by construction);
    band + boundary masks are multiplicative 0/1 applied after exp.
  - attn rows are normalized, then transposed via the PE; attended is
    computed directly in transposed form attT [E, q] so the output
    projection needs no further transpose. v's bias is folded in as a
    per-partition bias on the attT copy (sum of normalized attn rows = 1).
  - all matmuls run in float32r (fp32 with 12 low mantissa bits rounded
    away; full PE speed at moving-dim >= 256, ~1.5e-4 rms rel error).

The full-precision fallback (MM_DT=f32) can be selected with env
LWA_MM_F32=1 before first call (4x PE cost on matmuls).
"""

import os
import sys

sys.path.insert(0, "/opt/trn_rl_repo")

import numpy as np

import concourse.bass as bass  # noqa: F401  (registers types)
import concourse.tile as tile
from concourse import bacc, mybir
from concourse.bass_utils import run_bass_kernel_spmd

F32 = mybir.dt.float32
F32R = mybir.dt.float32r

B, S, E = 4, 4096, 512
WINDOW = 64
HALF = S // 2              # 2048 query rows per core
ROWS = HALF + 2 * WINDOW   # 2176 local rows incl. halo
EC = E // 128              # 4 contraction chunks
NT = HALF // 128           # 16 query subtiles per core
NDT = NT // 2              # 8 double tiles
# row slices for the qkv projection moving dim (all >= 256 for f32r speed)
RSLICES = [(0, 512), (512, 512), (1024, 512), (1536, 384), (1920, 256)]

_NC_CACHE = {}


def _round_fp32r(x: np.ndarray) -> np.ndarray:
    """Round-to-nearest fp32 -> fp32r (11-bit mantissa) as walrus expects."""
    u = x.view(np.uint32)
    r = (u.astype(np.uint64) + 0x800) & 0xFFFFF000
    return np.ascontiguousarray(r.astype(np.uint32).view(np.float32))


def _build(mm_f32: bool):
    MM = F32 if mm_f32 else F32R
    nc = bacc.Bacc("TRN2", target_bir_lowering=False, debug=False, num_devices=8)

    xT_d = nc.dram_tensor("xT", [E, ROWS], MM, kind="ExternalInput")
    wqkv_d = nc.dram_tensor("wqkv", [E, 3 * E], MM, kind="ExternalInput")
    bqkv_d = nc.dram_tensor("bqkv", [128, 12], F32, kind="ExternalInput")
    wout_d = nc.dram_tensor("wout", [E, E], MM, kind="ExternalInput")
    bout_d = nc.dram_tensor("bout", [1, E], MM, kind="ExternalInput")
    ones_d = nc.dram_tensor("ones", [1, 128], MM, kind="ExternalInput")
    mask_d = nc.dram_tensor("masks", [128, 3 * 256], F32, kind="ExternalInput")
    zero_d = nc.dram_tensor("zeros", [128, 256], MM, kind="ExternalInput")
    id_d = nc.dram_tensor("ident", [128, 128], F32, kind="ExternalInput")
    out_d = nc.dram_tensor("out", [HALF, E], F32, kind="ExternalOutput")

    ACT = mybir.ActivationFunctionType
    ALU = mybir.AluOpType

    with tile.TileContext(nc) as tc:
        with (
            tc.tile_pool(name="const", bufs=1) as const,
            tc.tile_pool(name="big", bufs=1) as big,
        ):
            # ---- constants ----
            wq_sb = [const.tile([128, 3 * E], MM, name=f"wq{e}", tag=f"wq{e}")
                     for e in range(EC)]
            wo_sb = [const.tile([128, E], MM, name=f"wo{e}", tag=f"wo{e}")
                     for e in range(EC)]
            bq_sb = const.tile([128, 12], F32, name="bq", tag="bq")
            bo_sb = const.tile([1, E], MM, name="bo", tag="bo")
            ones_sb = const.tile([1, 128], MM, name="ones1", tag="ones1")
            mask_sb = const.tile([128, 3 * 256], F32, name="msk", tag="msk")
            id_sb = const.tile([128, 128], F32, name="idn", tag="idn")
            # W_qkv loads split by purpose (q first so projection can
            # start as soon as the first xT row-slices land)
            for e in range(EC):
                nc.sync.dma_start(out=wq_sb[e][:, 0:E],
                                  in_=wqkv_d[128 * e:128 * (e + 1), 0:E])
            nc.sync.dma_start(out=bq_sb, in_=bqkv_d[:, :])

            # ---- persistent products ----
            qkT = [big.tile([128, ROWS], MM, name=f"qkT{f}", tag=f"qkT{f}")
                   for f in range(8)]          # f 0..3 = qT chunks, 4..7 = kT
            v_sb = [big.tile([128, E], MM, name=f"v{r}", tag=f"v{r}")
                    for r in range(ROWS // 128)]   # 17 natural-layout v chunks

            # ---- phase 1: projections (xT pool scoped so attention reuses
            #      its SBUF zone) ----
            with (
                tc.tile_pool(name="xTp", bufs=1) as xTp,
                tc.tile_pool(name="pp", bufs=4, space="PSUM") as pp,
            ):
                xT = [xTp.tile([128, ROWS], MM, name=f"xT{e}", tag=f"xT{e}")
                      for e in range(EC)]
                for si, (r0, ns) in enumerate(RSLICES):
                    for e in range(EC):
                        nc.sync.dma_start(
                            out=xT[e][:, r0:r0 + ns],
                            in_=xT_d[128 * e:128 * (e + 1), r0:r0 + ns])
                    if si == 0:  # k-projection weights after first slice
                        for e in range(EC):
                            nc.sync.dma_start(
                                out=wq_sb[e][:, E:2 * E],
                                in_=wqkv_d[128 * e:128 * (e + 1), E:2 * E])
                # late-phase constants
                for e in range(EC):
                    nc.sync.dma_start(out=wq_sb[e][:, 2 * E:3 * E],
                                      in_=wqkv_d[128 * e:128 * (e + 1), 2 * E:3 * E])
                for e in range(EC):
                    nc.sync.dma_start(out=wo_sb[e],
                                      in_=wout_d[128 * e:128 * (e + 1), :])
                nc.sync.dma_start(out=bo_sb, in_=bout_d[:, :])
                nc.sync.dma_start(out=ones_sb, in_=ones_d[:, :])
                nc.sync.dma_start(out=mask_sb, in_=mask_d[:, :])
                nc.sync.dma_start(out=id_sb, in_=id_d[:, :])

                # qT / kT: [feature, rows]
                for f in range(8):
                    for (r0, ns) in RSLICES:
                        ps = pp.tile([128, 512], F32, name=f"pq{f}_{r0}", tag="pp")
                        for e in range(EC):
                            nc.tensor.matmul(
                                ps[:, :ns],
                                wq_sb[e][:, 128 * f:128 * (f + 1)],
                                xT[e][:, r0:r0 + ns],
                                start=(e == 0), stop=(e == EC - 1),
                            )
                        nc.scalar.activation(
                            out=qkT[f][:, r0:r0 + ns], in_=ps[:, :ns],
                            func=ACT.Identity, bias=bq_sb[:, f:f + 1],
                        )

                # v: [rows, feature] (bias folded into attT copy later)
                for r in range(ROWS // 128):
                    ps = pp.tile([128, 512], F32, name=f"pv{r}", tag="pp")
                    for e in range(EC):
                        nc.tensor.matmul(
                            ps[:],
                            xT[e][:, 128 * r:128 * (r + 1)],
                            wq_sb[e][:, 2 * E:3 * E],
                            start=(e == 0), stop=(e == EC - 1),
                        )
                    nc.vector.tensor_copy(v_sb[r][:], ps[:])

            # ---- phase 2: attention + output projection ----
            with (
                tc.tile_pool(name="attn", bufs=2) as attn,
                tc.tile_pool(name="ptp", bufs=1) as ptp,
                tc.tile_pool(name="ps_s", bufs=2, space="PSUM") as ps_s,
                tc.tile_pool(name="ps_t", bufs=2, space="PSUM") as ps_t,
                tc.tile_pool(name="ps_a", bufs=2, space="PSUM") as ps_a,
                tc.tile_pool(name="pp_out", bufs=2, space="PSUM") as pp_out,
            ):
                # pT0 right half / pT2 left half stay zero for the whole
                # kernel (bufs=1, written halves only)
                pT0 = ptp.tile([128, 256], MM, name="pT0", tag="pT0")
                pT2 = ptp.tile([128, 256], MM, name="pT2", tag="pT2")
                nc.sync.dma_start(out=pT0[:], in_=zero_d[:, :])
                nc.sync.dma_start(out=pT2[:], in_=zero_d[:, :])

                for T in range(NDT):
                    pT1 = attn.tile([128, 256], MM, name=f"pT1_{T}", tag="pT1")
                    for s_half in (0, 1):
                        t = 2 * T + s_half
                        # scores [128 q, 256 keys]
                        ps = ps_s.tile([128, 256], F32, name=f"s{t}", tag="ps_s")
                        for e in range(EC):
                            nc.tensor.matmul(
                                ps[:],
                                qkT[e][:, 64 + 128 * t:192 + 128 * t],
                                qkT[4 + e][:, 128 * t:128 * t + 256],
                                start=(e == 0), stop=(e == EC - 1),
                            )
                        # additive band mask (0 / -1e30), exp with fused
                        # row-sum, then normalize into a fresh tile
                        mi = 0 if t == 0 else (2 if t == NT - 1 else 1)
                        sm = attn.tile([128, 256], F32, name=f"sm{t}", tag="sm")
                        nc.vector.tensor_add(
                            sm[:], ps[:], mask_sb[:, 256 * mi:256 * (mi + 1)])
                        pe_t = attn.tile([128, 256], F32, name=f"pe{t}", tag="pe")
                        rs = attn.tile([128, 1], F32, name=f"rs{t}", tag="rs")
                        nc.scalar.activation(out=pe_t[:], in_=sm[:], func=ACT.Exp,
                                             accum_out=rs[:])
                        rd = attn.tile([128, 1], F32, name=f"rd{t}", tag="rd")
                        nc.vector.reciprocal(rd[:], rs[:])
                        p_t = attn.tile([128, 256], F32, name=f"p{t}", tag="p")
                        nc.vector.tensor_scalar_mul(p_t[:], pe_t[:], rd[:])
                        # transpose both halves onto pT tiles
                        for half in (0, 1):
                            pt_ps = ps_t.tile([128, 128], F32,
                                              name=f"tp{t}_{half}", tag="ps_t")
                            nc.tensor.transpose(
                                pt_ps[:], p_t[:, 128 * half:128 * (half + 1)],
                                id_sb[:])
                            if s_half == 0 and half == 0:
                                dst = pT0[:, 0:128]
                            elif s_half == 0 and half == 1:
                                dst = pT1[:, 0:128]
                            elif s_half == 1 and half == 0:
                                dst = pT1[:, 128:256]
                            else:
                                dst = pT2[:, 128:256]
                            nc.vector.tensor_copy(dst, pt_ps[:])

                    # attended, transposed: attT[e', q(256)]
                    pTs = (pT0, pT1, pT2)
                    attT = []
                    for e in range(EC):
                        pa = ps_a.tile([128, 256], F32, name=f"pa{T}_{e}", tag="ps_a")
                        for kc in range(3):
                            nc.tensor.matmul(
                                pa[:],
                                v_sb[2 * T + kc][:, 128 * e:128 * (e + 1)],
                                pTs[kc][:],
                                start=(kc == 0), stop=(kc == 2),
                            )
                        at = attn.tile([128, 256], MM, name=f"attT{T}_{e}",
                                       tag=f"attT{e}")
                        nc.scalar.activation(
                            out=at[:], in_=pa[:],
                            func=ACT.Identity, bias=bq_sb[:, 8 + e:9 + e],
                        )
                        attT.append(at)

                    # output projection per 128-query subtile
                    for s_half in (0, 1):
                        t = 2 * T + s_half
                        po = pp_out.tile([128, 512], F32, name=f"po{t}", tag="pp_out")
                        for e in range(EC):
                            nc.tensor.matmul(
                                po[:],
                                attT[e][:, 128 * s_half:128 * (s_half + 1)],
                                wo_sb[e][:],
                                start=(e == 0), stop=False,
                            )
                        nc.tensor.matmul(
                            po[:], ones_sb[:], bo_sb[:], start=False, stop=True,
                        )
                        ost = attn.tile([128, 512], F32, name=f"ost{t}", tag="ost")
                        nc.vector.tensor_copy(ost[:], po[:])
                        nc.sync.dma_start(
                            out=out_d[128 * t:128 * (t + 1), :], in_=ost[:])
    nc.compile()
    return nc


def _get_nc():
    mm_f32 = bool(int(os.environ.get("LWA_MM_F32", "0")))
    key = ("nc", mm_f32)
    if key not in _NC_CACHE:
        _NC_CACHE[key] = _build(mm_f32)
    return _NC_CACHE[key], mm_f32


def _prep_shared(W_qkv, b_qkv, W_out, b_out, mm_f32):
    rnd = (lambda a: np.ascontiguousarray(a)) if mm_f32 else _round_fp32r
    scale = 1.0 / np.sqrt(np.float32(E))
    w = np.array(W_qkv, dtype=np.float32, copy=True)
    w[:, :E] *= scale
    b = np.array(b_qkv, dtype=np.float32, copy=True)
    b[:E] *= scale
    shared = {
        "wqkv": rnd(w),
        "bqkv": np.ascontiguousarray(b.reshape(12, 128).T),
        "wout": rnd(np.array(W_out, dtype=np.float32)),
        "bout": rnd(np.array(b_out, dtype=np.float32).reshape(1, E)),
        "ones": rnd(np.ones((1, 128), dtype=np.float32)),
        "zeros": np.zeros((128, 256), dtype=np.float32),
        "ident": np.eye(128, dtype=np.float32),
    }
    return shared


def _masks_for(h: int) -> np.ndarray:
    """Additive masks: 0 where attendable, -1e30 outside the band (or past
    the sequence boundary). Columns: [t0 mask | interior mask | t15 mask]."""
    ii = np.arange(128)[:, None]
    jj = np.arange(256)[None, :]
    band = (jj - ii >= 0) & (jj - ii <= 2 * WINDOW)
    m_mid = band
    m_t0 = band & (jj >= 64) if h == 0 else band
    m_t15 = band & (jj < 192) if h == 1 else band
    stacked = np.concatenate([m_t0, m_mid, m_t15], axis=1)
    return np.ascontiguousarray(
        np.where(stacked, np.float32(0.0), np.float32(-1e30)))


def _install_ntff_shim():
    """The agent image's antenv lacks axon_hooks; synthesize it from the
    boot module's ctypes NTFF driver so trace=True can capture HW timing."""
    import types
    if "antenv.axon_hooks" in sys.modules:
        return
    try:
        from trn_agent_boot.trn_boot import _ntff_profile_via_ctypes
        hook = _ntff_profile_via_ctypes("/opt/axon/libaxon_pjrt.so")
    except Exception:
        hook = None
    mod = types.ModuleType("antenv.axon_hooks")
    mod.get_axon_ntff_profile_hook = lambda: hook
    mod.set_axon_ntff_profile_hook = lambda h: None
    sys.modules["antenv.axon_hooks"] = mod
    # avoid S3 artifact upload attempts during local profile processing
    try:
        from concourse import bass_utils as _bu
        _bu.upload_artifacts = lambda tmpdir: tmpdir
    except Exception:
        pass


def kernel(x, W_qkv, b_qkv, W_out, b_out, _trace=False):
    x = np.asarray(x, dtype=np.float32)
    nc, mm_f32 = _get_nc()
    rnd = (lambda a: np.ascontiguousarray(a)) if mm_f32 else _round_fp32r
    shared = _prep_shared(W_qkv, b_qkv, W_out, b_out, mm_f32)
    masks = [_masks_for(0), _masks_for(1)]

    in_maps = []
    for core in range(8):
        b, h = divmod(core, 2)
        lo = h * HALF - WINDOW
        hi = lo + ROWS
        xh = np.zeros((ROWS, E), dtype=np.float32)
        s0, s1 = max(lo, 0), min(hi, S)
        xh[s0 - lo:s1 - lo] = x[b, s0:s1]
        in_maps.append({
            "xT": rnd(np.ascontiguousarray(xh.T)),
            "masks": masks[h],
            **shared,
        })

    kwargs = {}
    if _trace:
        _install_ntff_shim()
        kwargs = dict(trace=True, trace_cores=[0])
    res = run_bass_kernel_spmd(nc, in_maps, core_ids=list(range(8)), **kwargs)

    out = np.empty((B, S, E), dtype=np.float32)
    for core in range(8):
        b, h = divmod(core, 2)
        out[b, h * HALF:(h + 1) * HALF] = res.results[core]["out"]
    if _trace:
        return out, res
    return out

